# revision 37
# baseline (speedup 1.0000x reference)
"""TRN2 Bass kernel for nn_CustomLinear_66005057405513.

Computes y = FFT_4096(w * x)[:, :3072] for x: [4096, 4096] complex64
(given as interleaved float pairs) and w: [4096] complex64 twiddles.

Strategy: data-parallel over 8 NeuronCores (512 batch rows each). On each
core, a two-step radix-64 FFT with all twiddles folded into precomputed
matrices, processed slab-by-slab (128 batch rows) so every phase's
PSUM->SBUF copies trail a long in-order PE chain:

  n = 64*o + i, k = p + 64*q, q < 48:
    stage 1 (per i):  A[b, i, p] = sum_o C1[i][o, p] * x[b, 64o+i]
    stage 2 (per p):  y[b, p+64q] = sum_i C2[p][i, q] * A[b, i, p]

v7 (vs the v1 baseline): x is host-pre-transposed to [(o,c), (s, i, b')]
so stage-1 needs NO PE transposes - each (slab, i) is one matmul with the
x tile as the stationary operand (lhsT) and pairmat(C1[i]) streaming,
writing A[b, (p,c)] directly. Stage 2 is unchanged from v1: PE corner
turn [b,(i,c)] -> [(i,c),b] + per-p matmuls (N=96). PE op count drops
from 1024 to 768 per iteration.
"""

import numpy as np

import concourse.bass as bass
import concourse.mybir as mybir
from concourse import bacc
from concourse.tile import TileContext
from concourse.masks import make_identity
from concourse.bass_utils import run_bass_kernel_spmd

O = I = 64
N_FFT = O * I          # 4096
Q = 48                 # q < 48  <=>  k < 3072
OUT_F = Q * O * 2      # 6144 floats per output row
B_TOTAL = 4096
N_CORES = 8
B_LOCAL = B_TOTAL // N_CORES  # 512
SLABS = B_LOCAL // 128

UNROLL_JOBS = 4


def _make_tables(w_complex):
    oo = np.arange(O)
    W64 = np.exp(-2j * np.pi * np.outer(oo, oo) / O)
    WN = np.exp(-2j * np.pi * np.outer(np.arange(I), oo) / N_FFT)

    def pairmat(C):
        K, M = C.shape
        G = np.empty((2 * K, 2 * M), np.float64)
        G[0::2, 0::2] = C.real
        G[1::2, 0::2] = -C.imag
        G[0::2, 1::2] = C.imag
        G[1::2, 1::2] = C.real
        return G

    g1 = np.empty((128, I, 128), np.float64)
    for i in range(I):
        C1 = W64 * w_complex[64 * oo + i][:, None]
        g1[:, i, :] = pairmat(C1)
    g2 = np.empty((128, O, 96), np.float64)
    for p in range(O):
        C2 = WN[:, p][:, None] * W64[:, :Q]
        g2[:, p, :] = pairmat(C2)
    return g1, g2


def _build_nc(compute="f16", act_every=2, reps=1, unroll=False):
    f32 = mybir.dt.float32
    cdt = mybir.dt.float16
    TP = 1024
    G1T = TP // 128  # 8

    nc = bacc.Bacc(None, target_bir_lowering=False, debug=False)
    x = nc.declare_dram_parameter("x", [128, SLABS * I * 128], cdt,
                                  isOutput=False)
    w1 = nc.declare_dram_parameter("w1", [128, I * 128], cdt, isOutput=False)
    w2 = nc.declare_dram_parameter("w2", [128, O * 96], cdt, isOutput=False)
    y = nc.declare_dram_parameter("y", [B_LOCAL, OUT_F], cdt, isOutput=True)

    cc = [0]

    def copy(out_ap, in_ap):
        cc[0] += 1
        if not act_every or cc[0] % act_every:
            nc.vector.tensor_copy(out_ap, in_ap)
        else:
            nc.scalar.copy(out_ap, in_ap)

    with TileContext(nc) as tc:
        with (
            tc.tile_pool(name="const", bufs=1) as cpool,
            tc.tile_pool(name="xp", bufs=3) as xpool,
            tc.tile_pool(name="ap", bufs=2) as apool,
            tc.tile_pool(name="yp", bufs=2) as ypool,
            tc.tile_pool(name="tp", bufs=4) as tpool,
            tc.tile_pool(name="pt1", bufs=2, space="PSUM") as pt1,
            tc.tile_pool(name="pm1", bufs=6, space="PSUM") as pm1,
        ):
            ident = cpool.tile([128, 128], cdt, name="ident")
            make_identity(nc, ident[:])
            w1s = cpool.tile([128, I * 128], cdt, name="w1s")
            nc.scalar.dma_start(out=w1s[:], in_=w1[:])
            w2s = cpool.tile([128, O * 96], cdt, name="w2s")
            nc.scalar.dma_start(out=w2s[:], in_=w2[:])
            w1v = w1s[:].rearrange("k (i n) -> k i n", i=I)
            w2v = w2s[:].rearrange("k (p n) -> k p n", p=O)

            def job(_iv=None):
                xs_t, ab_t, yb_t, t2sg = {}, {}, {}, {}

                def load_x(s):
                    xs = xpool.tile([128, 8192], cdt, name="xs")
                    for ch in range(2):
                        nc.sync.dma_start(
                            out=xs[:, ch * 4096:(ch + 1) * 4096],
                            in_=x[:, s * 8192 + ch * 4096:
                                  s * 8192 + (ch + 1) * 4096])
                    xs_t[s] = xs

                def s1_group(s, g):
                    if g == 0:
                        ab_t[s] = apool.tile([128, 8192], cdt, name="Ab")
                    m1p = pm1.tile([128, 512], f32, name="m1p")
                    xs = xs_t[s]
                    for j in range(4):
                        i = g * 4 + j
                        nc.tensor.matmul(
                            m1p[:, j * 128:(j + 1) * 128],
                            lhsT=xs[:, i * 128:(i + 1) * 128],
                            rhs=w1v[:, i, :], start=True, stop=True)
                    av_w = ab_t[s][:].rearrange("b (p i c) -> b i p c",
                                                p=O, c=2)
                    copy(av_w[:, g * 4:g * 4 + 4, :, :],
                         m1p[:].rearrange("b (i p c) -> b i p c", c=2, i=4))

                def s2_turn(s, g):
                    if g == 0:
                        yb_t[s] = ypool.tile([128, OUT_F], cdt, name="Yb")
                    t2p = pt1.tile([128, TP], cdt, name="t1p")
                    Ab = ab_t[s]
                    for j in range(G1T):
                        p = g * G1T + j
                        nc.tensor.transpose(
                            t2p[:, j * 128:(j + 1) * 128],
                            Ab[:, p * 128:(p + 1) * 128], ident[:])
                    t2s = tpool.tile([128, TP], cdt, name="t2s")
                    copy(t2s[:], t2p[:])
                    t2sg[(s, g)] = t2s

                def s2_mm(s, g):
                    t2s = t2sg.pop((s, g))
                    yv = yb_t[s][:].rearrange("b (q p c) -> b p q c",
                                              p=O, c=2)
                    for h in range(G1T // 4):
                        m2p = pm1.tile([128, 512], f32, name="m1p")[:, :4 * 96]
                        for j in range(4):
                            jj = h * 4 + j
                            p = g * G1T + jj
                            nc.tensor.matmul(
                                m2p[:, j * 96:(j + 1) * 96],
                                lhsT=t2s[:, jj * 128:(jj + 1) * 128],
                                rhs=w2v[:, p, :], start=True, stop=True)
                        p0 = g * G1T + h * 4
                        copy(yv[:, p0:p0 + 4, :, :],
                             m2p[:].rearrange("b (p q c) -> b p q c",
                                              q=Q, c=2))

                NG2 = O // G1T  # 8 s2 groups per slab
                load_x(0)
                for g in range(I // 4):
                    s1_group(0, g)
                if SLABS > 1:
                    load_x(1)
                for s in range(SLABS):
                    if s + 2 < SLABS:
                        load_x(s + 2)  # two slabs of DMA lead
                    # interleave slab s stage-2 with slab s+1 stage-1 so
                    # the PE never idles on the Ab / t2s copy boundaries
                    for step in range(NG2 + 1):
                        if step < NG2:
                            s2_turn(s, step)
                        if s + 1 < SLABS:
                            for gg in (2 * step, 2 * step + 1):
                                if gg < I // 4:
                                    s1_group(s + 1, gg)
                        if step >= 1:
                            s2_mm(s, step - 1)
                    nc.scalar.dma_start(out=y[s * 128:(s + 1) * 128, :],
                                        in_=yb_t.pop(s)[:])

            if reps > 1 and unroll:
                for _ in range(reps):
                    job()
            elif reps > 1:
                U = UNROLL_JOBS
                if U > 1:
                    assert (reps - 1) % U == 0, (reps, U)
                    job()
                    with tc.For_i(0, (reps - 1) // U, 1) as _i:
                        for _ in range(U):
                            job()
                else:
                    with tc.For_i(0, reps, 1) as _i:
                        job(_i)
            else:
                job()

    nc.compile()
    return nc


_NC_CACHE = {}


def _get_nc(compute="f16"):
    if compute not in _NC_CACHE:
        _NC_CACHE[compute] = _build_nc(compute)
    return _NC_CACHE[compute]


def _host_inputs(x_real, weights_real, compute="f16"):
    np_dt = np.float16
    wr = np.asarray(weights_real, dtype=np.float64)
    wc = wr[0::2] + 1j * wr[1::2]
    g1, g2 = _make_tables(wc)
    w1 = np.ascontiguousarray(g1.reshape(128, -1)).astype(np_dt)
    w2 = np.ascontiguousarray(g2.reshape(128, -1)).astype(np_dt)
    x = np.asarray(x_real)
    # [core, s, b', o, i, c] -> [core, o, c, s, i, b']
    xf = x.reshape(N_CORES, SLABS, 128, O, I, 2).transpose(0, 3, 5, 1, 4, 2)
    xf = np.ascontiguousarray(xf).reshape(N_CORES, 128, -1).astype(np_dt)
    return [{"x": xf[c], "w1": w1, "w2": w2} for c in range(N_CORES)]


def kernel(x_real, weights_real):
    nc = _get_nc()
    in_maps = _host_inputs(x_real, weights_real)
    res = run_bass_kernel_spmd(nc, in_maps, list(range(N_CORES)))
    outs = []
    for c in range(N_CORES):
        v = np.asarray(res.results[c]["y"], dtype=np.float32)
        v = v.reshape(B_LOCAL, Q * O, 2)
        outs.append((v[..., 0] + 1j * v[..., 1]).astype(np.complex64))
    return np.concatenate(outs, axis=0)
